# revision 5
# baseline (speedup 1.0000x reference)
"""Multi-head attention (B=2, S=2048, D=1024, H=16) on 8 TRN2 NeuronCores.

Sharding: data parallel on batch (2) x tensor parallel on heads (4 groups of
4 heads).  Core c handles batch c//4, heads 4*(c%4) .. 4*(c%4)+4.  Each core
computes q/k/v projections for its 256 output dims, attention for its 4
heads, and a partial (row-parallel) output projection.  The host sums the 4
partials per batch and adds b_o.

v2 schedule (trace-driven rework of the v1 kernel):
  - Steady-state is co-paced: ScalarE exp of [128,1024] is 1107ns/jt; the PE
    chain (scores pair 216 + 2xPV 432 + ~1 filler) is ~1050-1250ns/jt.  The
    v1 fat was: 26.6us DMA/cold head, +15us PE-oversubscribed phase 0,
    +5.8us phase 1, ~12us of mid-body ACT gaps from bursty fillers, and a
    ~27us serialized HAM-throttled o_proj tail.
  - Head: DMA order wk, x-c0, wq, wv, ... so kT(0,0)/qT(0,0)/v(0..3) all run
    inside the DMA window; first ACT fires at ~10us instead of 26.6us.
  - Fillers are split into ~2-matmul "parts" placed per jt slot with
    deadlines: kT chunks just before the scores that read them, v groups
    allowed to lag via the 12-deep E pool (PV consumes E one jt late, so the
    PE never blocks in-order on the exp semaphore: emission per jt is
    [PV(jt-1), fillers, scores(jt), ACT(jt)]).
  - o_proj spread over phases 3-7 as (st, half) parts; the last 4 s-tiles
    (ic3) emit as an interleaved dense tail through the freed Sp pool so the
    PE stays busy and HAM stays warm.
"""

import numpy as np
import ml_dtypes

B, S, D = 2, 2048, 1024
H, DH = 16, 64
N_CORES = 8
HPC = 4  # heads per core
DL = HPC * DH  # 256 local dims per core
KT = D // 128  # 8 k-tiles
ST = S // 128  # 16 s-tiles (also j-tiles)
IC = 512  # i-chunk (query chunk)
NIC = S // IC

_BF16 = ml_dtypes.bfloat16

_nc_cache = None


def _build_nc():
    from contextlib import ExitStack

    import concourse.mybir as mybir
    import concourse.tile as tile
    from concourse import bacc

    f32 = mybir.dt.float32
    bf16 = mybir.dt.bfloat16
    Alu = mybir.AluOpType
    Act = mybir.ActivationFunctionType

    nc = bacc.Bacc("TRN2", target_bir_lowering=False, debug=False, enable_asserts=False)

    xT_d = nc.dram_tensor("xT", (D, S), bf16, kind="ExternalInput")  # [k, s]
    wq_d = nc.dram_tensor("wq", (D, DL), bf16, kind="ExternalInput")  # [k, dl]
    wk_d = nc.dram_tensor("wk", (D, DL), bf16, kind="ExternalInput")
    wv_d = nc.dram_tensor("wv", (D, DL), bf16, kind="ExternalInput")
    wo_d = nc.dram_tensor("wo", (DL, D), bf16, kind="ExternalInput")  # [dl, o]
    bqk_d = nc.dram_tensor("bqk", (128, 4), f32, kind="ExternalInput")
    bv_d = nc.dram_tensor("bv", (128, DL), f32, kind="ExternalInput")
    out_d = nc.dram_tensor("out", (S, D), f32, kind="ExternalOutput")

    with tile.TileContext(nc) as tc, ExitStack() as ctx:
        consts = ctx.enter_context(tc.tile_pool(name="consts", bufs=1))
        xbf = consts.tile([128, KT, S], bf16)  # [p, kt, s]
        wq_sb = consts.tile([128, KT, DL], bf16)
        wk_sb = consts.tile([128, KT, DL], bf16)
        wv_sb = consts.tile([128, KT, DL], bf16)
        wo_sb = consts.tile([128, 2, D], bf16)  # [p, kt2, o]
        bqk_sb = consts.tile([128, 4], f32)
        bv_sb = consts.tile([128, DL], f32)
        qT = consts.tile([128, 2, S], bf16)  # [p, mt(pair), s]
        kT = consts.tile([128, 2, S], bf16)
        # v (s-major) + ones column at 64, zero-padded to 128 cols (full-M PV)
        vaug = consts.tile([128, ST, HPC, 128], bf16)  # [p(j), jt, h, dd]
        aoT = consts.tile([128, 2, S], bf16)  # attn-out transposed [p, kt2, s]

        # Preload the exp activation table set (~2.7us) immediately.
        warm = consts.tile([128, 8], f32)
        nc.gpsimd.memset(warm[:], 0.0)
        nc.scalar.activation(warm[:], warm[:], Act.Exp)
        nc.gpsimd.memset(vaug[:, :, :, DH + 1 :], 0.0)
        nc.gpsimd.memset(vaug[:, :, :, DH : DH + 1], 1.0)

        # ---- input DMAs, ordered for the head-stage compute:
        # wk + x chunk 0 gate kT(0,0); wq gates qT(0,0); wv/bv gate v(0..3).
        nc.sync.dma_start(wk_sb[:], wk_d.ap().rearrange("(kt p) m -> p kt m", p=128))
        for kt in range(KT):
            nc.sync.dma_start(
                xbf[:, kt, 0:512], xT_d.ap()[kt * 128 : (kt + 1) * 128, 0:512]
            )
        nc.sync.dma_start(wq_sb[:], wq_d.ap().rearrange("(kt p) m -> p kt m", p=128))
        nc.sync.dma_start(bqk_sb[:], bqk_d.ap())
        nc.sync.dma_start(wv_sb[:], wv_d.ap().rearrange("(kt p) m -> p kt m", p=128))
        nc.sync.dma_start(bv_sb[:], bv_d.ap())
        for sc in range(1, 4):
            for kt in range(KT):
                nc.sync.dma_start(
                    xbf[:, kt, sc * 512 : (sc + 1) * 512],
                    xT_d.ap()[kt * 128 : (kt + 1) * 128, sc * 512 : (sc + 1) * 512],
                )
        nc.sync.dma_start(wo_sb[:], wo_d.ap().rearrange("(kt p) m -> p kt m", p=128))

        ps = ctx.enter_context(tc.tile_pool(name="ps", bufs=2, space="PSUM"))
        op_ = ctx.enter_context(tc.tile_pool(name="op", bufs=3, space="PSUM"))
        fp = ctx.enter_context(tc.tile_pool(name="fp", bufs=1, space="PSUM"))
        ep = ctx.enter_context(tc.tile_pool(name="ep", bufs=12))
        rp = ctx.enter_context(tc.tile_pool(name="rp", bufs=3))
        tp = ctx.enter_context(tc.tile_pool(name="tp", bufs=3))
        osb = ctx.enter_context(tc.tile_pool(name="osb", bufs=3))

        # ---- filler groups, split into ~2-matmul parts (shared psum tile
        # per group; parts of one group must be emitted consecutively
        # relative to other fp-pool groups since fp has a single slot).
        def qk_group(proj, mt, c):
            """q (proj=0) / k (proj=1) projection of one 512-col chunk of
            head-pair mt, as 4 parts of 2 k-tiles each."""
            g = {}
            w_sb = wq_sb if proj == 0 else wk_sb

            def part(k0, k1):
                def f():
                    if "p" not in g:
                        g["p"] = fp.tile([128, 512], f32, tag="f", name="qkf")
                    p = g["p"]
                    for kt in range(k0, k1):
                        nc.tensor.matmul(
                            p[:],
                            w_sb[:, kt, mt * 128 : (mt + 1) * 128],
                            xbf[:, kt, c * 512 : (c + 1) * 512],
                            start=(kt == 0),
                            stop=(kt == KT - 1),
                        )
                    if k1 == KT:
                        dst = (qT if proj == 0 else kT)[:, mt, c * 512 : (c + 1) * 512]
                        bias_ap = bqk_sb[:, proj * 2 + mt : proj * 2 + mt + 1]
                        if proj == 0:
                            nc.vector.tensor_scalar(
                                dst, p[:], bias_ap, 0.125, Alu.add, Alu.mult
                            )
                        else:
                            nc.vector.tensor_scalar(dst, p[:], bias_ap, None, Alu.add)

                return f

            return [part(0, 2), part(2, 4), part(4, 6), part(6, 8)]

        def v_group(st):
            g = {}

            def part(k0, k1):
                def f():
                    if "p" not in g:
                        g["p"] = fp.tile([128, 512], f32, tag="f", name="vf")
                    p = g["p"]
                    for kt in range(k0, k1):
                        nc.tensor.matmul(
                            p[:, 0:DL],
                            xbf[:, kt, st * 128 : (st + 1) * 128],
                            wv_sb[:, kt, :],
                            start=(kt == 0),
                            stop=(kt == KT - 1),
                        )
                    if k1 == KT:
                        nc.vector.tensor_tensor(
                            vaug[:, st, :, 0:DH],
                            p[:, 0:DL].rearrange("p (h d) -> p h d", h=HPC),
                            bv_sb[:].rearrange("p (h d) -> p h d", h=HPC),
                            Alu.add,
                        )

                return f

            return [part(0, 2), part(2, 4), part(4, 6), part(6, 8)]

        def o_part(st, oc):
            """One 512-col chunk of the output projection for s-tile st:
            2 matmuls + evict + store, self-contained."""

            def f():
                pso = fp.tile([128, 512], f32, tag="f")
                for kt2 in range(2):
                    nc.tensor.matmul(
                        pso[:],
                        aoT[:, kt2, st * 128 : (st + 1) * 128],
                        wo_sb[:, kt2, oc * 512 : (oc + 1) * 512],
                        start=(kt2 == 0),
                        stop=(kt2 == 1),
                    )
                stg = osb.tile([128, 512], f32, tag="oh")
                nc.vector.tensor_copy(stg[:], pso[:])
                nc.sync.dma_start(
                    out_d.ap()[st * 128 : (st + 1) * 128, oc * 512 : (oc + 1) * 512],
                    stg[:],
                )

            return f

        def attn_norm(h, ic, O):
            pb, mt = 64 * (h % 2), h // 2
            den = rp.tile([1, IC], f32, tag="den")
            nc.vector.tensor_copy(den[:], O[DH : DH + 1, :])
            recip = rp.tile([1, IC], f32, tag="r")
            nc.vector.reciprocal_approx_fast(recip[:], den[:])
            rb = rp.tile([64, IC], f32, tag="rb")
            nc.gpsimd.partition_broadcast(rb[:], recip[:])
            tmp = tp.tile([64, IC], bf16, tag="t")
            nc.vector.tensor_tensor(tmp[:], O[0:DH, :], rb[:], Alu.mult)
            nc.sync.dma_start(aoT[pb : pb + 64, mt, ic * IC : (ic + 1) * IC], tmp[:])

        def pair_ic(pair, ic, fillers):
            """Attention for head pair (2*pair, 2*pair+1) on query chunk ic.
            fillers: {jt: [part, ...]} emitted inside that jt step.  PV runs
            one jt behind the exp so the in-order PE queue never waits on
            the ScalarE semaphore."""
            hA, hB = 2 * pair, 2 * pair + 1
            OA = op_.tile([128, IC], f32, tag="O")
            OB = op_.tile([128, IC], f32, tag="O")

            def pv(jt, E):
                nc.tensor.matmul(
                    OA[:], vaug[:, jt, hA, :], E[:, 0:IC],
                    start=(jt == 0), stop=(jt == ST - 1),
                )
                nc.tensor.matmul(
                    OB[:], vaug[:, jt, hB, :], E[:, IC : 2 * IC],
                    start=(jt == 0), stop=(jt == ST - 1),
                )

            prevE = None
            for jt in range(ST):
                Sp = ps.tile([128, 2 * IC], f32, tag="S")
                nc.tensor.matmul(
                    Sp[:, 0:IC],
                    kT[0:64, pair, jt * 128 : (jt + 1) * 128],
                    qT[0:64, pair, ic * IC : (ic + 1) * IC],
                    start=True, stop=True,
                )
                nc.tensor.matmul(
                    Sp[:, IC : 2 * IC],
                    kT[64:128, pair, jt * 128 : (jt + 1) * 128],
                    qT[64:128, pair, ic * IC : (ic + 1) * IC],
                    start=True, stop=True,
                )
                E = ep.tile([128, 2 * IC], bf16, tag="E")
                nc.scalar.activation(E[:], Sp[:], Act.Exp)
                if prevE is not None:
                    pv(jt - 1, prevE)
                for f in fillers.get(jt, ()):
                    f()
                prevE = E
            pv(ST - 1, prevE)
            attn_norm(hA, ic, OA)
            attn_norm(hB, ic, OB)

        # ---- emission schedule ----
        # Head (inside the DMA window): only kT(0,0) + qT(0,0) so the first
        # scores/exp fire as early as possible; everything else streams
        # through the jt filler slots.
        for f in qk_group(1, 0, 0):
            f()
        for f in qk_group(0, 0, 0):
            f()

        def slots(*assign):
            """assign: list of (slot, [parts...]) -> fillers dict."""
            d = {}
            for slot, parts in assign:
                d.setdefault(slot, []).extend(parts)
            return d

        K01, K02, K03 = qk_group(1, 0, 1), qk_group(1, 0, 2), qk_group(1, 0, 3)
        Q01 = qk_group(0, 0, 1)
        V = {st: v_group(st) for st in range(ST)}

        # phase 0 = (0,0): all 16 v groups (v(st) fully emitted by the end
        # of slot st, since PV(st) is emitted at slot st+1) + kT(0,1..3)
        # ahead of the scores that read them + qT(0,1) for phase 1.
        ph0 = slots(
            (0, V[0] + [V[1][0]]),
            (1, V[1][1:] + [K01[0], K01[1]]),
            (2, V[2] + [K01[2], K01[3]]),
            (3, V[3]),
            (4, V[4]),
            (5, V[5] + [K02[0]]),
            (6, V[6] + [K02[1], K02[2]]),
            (7, V[7] + [K02[3]]),
            (8, V[8]),
            (9, V[9] + [K03[0]]),
            (10, V[10] + [K03[1], K03[2]]),
            (11, V[11] + [K03[3]]),
            (12, V[12]),
            (13, V[13] + [Q01[0]]),
            (14, V[14] + [Q01[1], Q01[2]]),
            (15, V[15] + [Q01[3]]),
        )
        pair_ic(0, 0, ph0)

        K10, Q10 = qk_group(1, 1, 0), qk_group(0, 1, 0)
        # phase 1 = (0,1): kT(1,0) + qT(1,0) for phase 2.  Light.
        ph1 = slots(
            (0, [K10[0]]), (1, [K10[1]]), (2, [K10[2]]), (3, [K10[3]]),
            (4, [Q10[0]]), (5, [Q10[1]]), (6, [Q10[2]]), (7, [Q10[3]]),
        )
        pair_ic(0, 1, ph1)

        K11, K12, K13 = qk_group(1, 1, 1), qk_group(1, 1, 2), qk_group(1, 1, 3)
        Q11 = qk_group(0, 1, 1)
        # phase 2 = (1,0): remaining kT(1,*) just-in-time + qT(1,1).
        ph2 = slots(
            (0, [K11[0], K11[1]]),
            (1, [K11[2], K11[3]]),
            (4, [K12[0], K12[1]]),
            (5, [K12[2], K12[3]]),
            (8, [K13[0], K13[1]]),
            (9, [K13[2], K13[3]]),
            (12, [Q11[0]]), (13, [Q11[1]]), (14, [Q11[2]]), (15, [Q11[3]]),
        )
        pair_ic(1, 0, ph2)

        Q02 = qk_group(0, 0, 2)
        # phase 3 = (1,1): qT(0,2) + o_proj of ic0 s-tiles.
        ph3 = slots(
            (0, [Q02[0]]), (1, [Q02[1]]), (2, [Q02[2]]), (3, [Q02[3]]),
            (6, [o_part(0, 0)]), (7, [o_part(0, 1)]),
            (8, [o_part(1, 0)]), (9, [o_part(1, 1)]),
            (10, [o_part(2, 0)]), (11, [o_part(2, 1)]),
            (12, [o_part(3, 0)]), (13, [o_part(3, 1)]),
        )
        pair_ic(1, 1, ph3)

        Q12 = qk_group(0, 1, 2)
        ph4 = slots(
            (0, [Q12[0]]), (1, [Q12[1]]), (2, [Q12[2]]), (3, [Q12[3]]),
            (6, [o_part(4, 0)]), (7, [o_part(4, 1)]),
            (8, [o_part(5, 0)]), (9, [o_part(5, 1)]),
        )
        pair_ic(0, 2, ph4)

        Q03 = qk_group(0, 0, 3)
        ph5 = slots(
            (0, [Q03[0]]), (1, [Q03[1]]), (2, [Q03[2]]), (3, [Q03[3]]),
            (6, [o_part(6, 0)]), (7, [o_part(6, 1)]),
            (8, [o_part(7, 0)]), (9, [o_part(7, 1)]),
        )
        pair_ic(1, 2, ph5)

        Q13 = qk_group(0, 1, 3)
        ph6 = slots(
            (0, [Q13[0]]), (1, [Q13[1]]), (2, [Q13[2]]), (3, [Q13[3]]),
            (6, [o_part(8, 0)]), (7, [o_part(8, 1)]),
            (8, [o_part(9, 0)]), (9, [o_part(9, 1)]),
        )
        pair_ic(0, 3, ph6)

        ph7 = slots(
            (4, [o_part(10, 0)]), (5, [o_part(10, 1)]),
            (6, [o_part(11, 0)]), (7, [o_part(11, 1)]),
        )
        pair_ic(1, 3, ph7)

        # ---- dense tail: o_proj for s-tiles 12..15 through the freed Sp
        # pool (2x 2-bank tiles) so matmuls pipeline while evicts/DMAs run.
        def o_tail_mm(st):
            pso = ps.tile([128, 2 * IC], f32, tag="S")
            for n in range(2):
                for kt2 in range(2):
                    nc.tensor.matmul(
                        pso[:, n * 512 : (n + 1) * 512],
                        aoT[:, kt2, st * 128 : (st + 1) * 128],
                        wo_sb[:, kt2, n * 512 : (n + 1) * 512],
                        start=(kt2 == 0),
                        stop=(kt2 == 1),
                    )
            return pso

        def o_tail_evict(st, pso):
            for n in range(2):
                stg = osb.tile([128, 512], f32, tag="oh")
                nc.vector.tensor_copy(stg[:], pso[:, n * 512 : (n + 1) * 512])
                nc.sync.dma_start(
                    out_d.ap()[st * 128 : (st + 1) * 128, n * 512 : (n + 1) * 512],
                    stg[:],
                )

        p12 = o_tail_mm(12)
        p13 = o_tail_mm(13)
        o_tail_evict(12, p12)
        p14 = o_tail_mm(14)
        o_tail_evict(13, p13)
        p15 = o_tail_mm(15)
        o_tail_evict(14, p14)
        o_tail_evict(15, p15)

    nc.compile()
    return nc


def _get_nc():
    global _nc_cache
    if _nc_cache is None:
        _nc_cache = _build_nc()
    return _nc_cache


def _prepare_in_maps(x, W_q, b_q, W_k, b_k, W_v, b_v, W_o, b_o):
    in_maps = []
    for c in range(N_CORES):
        b, g = c // 4, c % 4
        rows = slice(DL * g, DL * g + DL)
        bqk = np.stack(
            [
                b_q[DL * g : DL * g + 128],
                b_q[DL * g + 128 : DL * g + 256],
                b_k[DL * g : DL * g + 128],
                b_k[DL * g + 128 : DL * g + 256],
            ],
            axis=1,
        ).astype(np.float32)
        in_maps.append(
            {
                "xT": np.ascontiguousarray(x[b].T).astype(_BF16),
                "wq": np.ascontiguousarray(W_q[rows].T).astype(_BF16),
                "wk": np.ascontiguousarray(W_k[rows].T).astype(_BF16),
                "wv": np.ascontiguousarray(W_v[rows].T).astype(_BF16),
                "wo": np.ascontiguousarray(W_o[:, rows].T).astype(_BF16),
                "bqk": np.ascontiguousarray(bqk),
                "bv": np.ascontiguousarray(
                    np.broadcast_to(b_v[rows], (128, DL))
                ).astype(np.float32),
            }
        )
    return in_maps


def _assemble(results, b_o):
    out = np.empty((B, S, D), dtype=np.float32)
    for b in range(B):
        acc = results[4 * b]["out"].astype(np.float32).copy()
        for g in range(1, 4):
            acc += results[4 * b + g]["out"]
        out[b] = acc + b_o[None, :].astype(np.float32)
    return out


def kernel(x, W_q, b_q, W_k, b_k, W_v, b_v, W_o, b_o):
    from concourse.bass_utils import run_bass_kernel_spmd

    x = np.asarray(x, dtype=np.float32)
    nc = _get_nc()
    in_maps = _prepare_in_maps(
        x,
        np.asarray(W_q, np.float32),
        np.asarray(b_q, np.float32),
        np.asarray(W_k, np.float32),
        np.asarray(b_k, np.float32),
        np.asarray(W_v, np.float32),
        np.asarray(b_v, np.float32),
        np.asarray(W_o, np.float32),
        np.asarray(b_o, np.float32),
    )
    res = run_bass_kernel_spmd(nc, in_maps, core_ids=list(range(N_CORES)))
    return _assemble(res.results, np.asarray(b_o, np.float32))


# revision 8
# speedup vs baseline: 1.0064x; 1.0064x over previous
"""Multi-head attention (B=2, S=2048, D=1024, H=16) on 8 TRN2 NeuronCores.

Sharding: data parallel on batch (2) x tensor parallel on heads (4 groups of
4 heads).  Core c handles batch c//4, heads 4*(c%4) .. 4*(c%4)+4.  Each core
computes q/k/v projections for its 256 output dims, attention for its 4
heads, and a partial (row-parallel) output projection.  The host sums the 4
partials per batch and adds b_o.

v2 schedule (trace-driven rework of the v1 kernel):
  - Steady-state is co-paced: ScalarE exp of [128,1024] is 1107ns/jt; the PE
    chain (scores pair 216 + 2xPV 432 + ~1 filler) is ~1050-1250ns/jt.  The
    v1 fat was: 26.6us DMA/cold head, +15us PE-oversubscribed phase 0,
    +5.8us phase 1, ~12us of mid-body ACT gaps from bursty fillers, and a
    ~27us serialized HAM-throttled o_proj tail.
  - Head: DMA order wk, x-c0, wq, wv, ... so kT(0,0)/qT(0,0)/v(0..3) all run
    inside the DMA window; first ACT fires at ~10us instead of 26.6us.
  - Fillers are split into ~2-matmul "parts" placed per jt slot with
    deadlines: kT chunks just before the scores that read them, v groups
    allowed to lag via the 12-deep E pool (PV consumes E one jt late, so the
    PE never blocks in-order on the exp semaphore: emission per jt is
    [PV(jt-1), fillers, scores(jt), ACT(jt)]).
  - o_proj spread over phases 3-7 as (st, half) parts; the last 4 s-tiles
    (ic3) emit as an interleaved dense tail through the freed Sp pool so the
    PE stays busy and HAM stays warm.
"""

import numpy as np
import ml_dtypes

B, S, D = 2, 2048, 1024
H, DH = 16, 64
N_CORES = 8
HPC = 4  # heads per core
DL = HPC * DH  # 256 local dims per core
KT = D // 128  # 8 k-tiles
ST = S // 128  # 16 s-tiles (also j-tiles)
IC = 512  # i-chunk (query chunk)
NIC = S // IC

_BF16 = ml_dtypes.bfloat16

_nc_cache = None


def _build_nc():
    from contextlib import ExitStack

    import concourse.mybir as mybir
    import concourse.tile as tile
    from concourse import bacc

    f32 = mybir.dt.float32
    bf16 = mybir.dt.bfloat16
    Alu = mybir.AluOpType
    Act = mybir.ActivationFunctionType

    nc = bacc.Bacc("TRN2", target_bir_lowering=False, debug=False, enable_asserts=False)

    xT_d = nc.dram_tensor("xT", (D, S), bf16, kind="ExternalInput")  # [k, s]
    wq_d = nc.dram_tensor("wq", (D, DL), bf16, kind="ExternalInput")  # [k, dl]
    wk_d = nc.dram_tensor("wk", (D, DL), bf16, kind="ExternalInput")
    wv_d = nc.dram_tensor("wv", (D, DL), bf16, kind="ExternalInput")
    wo_d = nc.dram_tensor("wo", (DL, D), bf16, kind="ExternalInput")  # [dl, o]
    bqk_d = nc.dram_tensor("bqk", (128, 4), f32, kind="ExternalInput")
    bv_d = nc.dram_tensor("bv", (128, DL), f32, kind="ExternalInput")
    out_d = nc.dram_tensor("out", (S, D), bf16, kind="ExternalOutput")

    with tile.TileContext(nc) as tc, ExitStack() as ctx:
        consts = ctx.enter_context(tc.tile_pool(name="consts", bufs=1))
        xbf = consts.tile([128, KT, S], bf16)  # [p, kt, s]
        wq_sb = consts.tile([128, KT, DL], bf16)
        wk_sb = consts.tile([128, KT, DL], bf16)
        wv_sb = consts.tile([128, KT, DL], bf16)
        wo_sb = consts.tile([128, 2, D], bf16)  # [p, kt2, o]
        bqk_sb = consts.tile([128, 4], f32)
        bv_sb = consts.tile([128, DL], f32)
        qT = consts.tile([128, 2, S], bf16)  # [p, mt(pair), s]
        kT = consts.tile([128, 2, S], bf16)
        # v (s-major) + ones column at 64, zero-padded to 128 cols (full-M PV)
        vaug = consts.tile([128, ST, HPC, 128], bf16)  # [p(j), jt, h, dd]
        aoT = consts.tile([128, 2, S], bf16)  # attn-out transposed [p, kt2, s]

        # Preload the exp activation table set (~2.7us) immediately.
        warm = consts.tile([128, 8], f32)
        nc.gpsimd.memset(warm[:], 0.0)
        nc.scalar.activation(warm[:], warm[:], Act.Exp)
        nc.gpsimd.memset(vaug[:, :, :, DH + 1 :], 0.0)
        nc.gpsimd.memset(vaug[:, :, :, DH : DH + 1], 1.0)

        # ---- input DMAs: few big transfers (the Sync queue issues DMAs
        # serially at ~0.7us each, so instruction count matters), ordered so
        # kT(0,0) [wk + x c0] and qT(0,0) [wq] can start earliest.
        def x_chunk(c, k0, k1):
            nc.sync.dma_start(
                xbf[:, k0:k1, c * 512 : (c + 1) * 512],
                xT_d.ap()[k0 * 128 : k1 * 128, c * 512 : (c + 1) * 512].rearrange(
                    "(kt p) s -> p kt s", p=128
                ),
            )

        nc.sync.dma_start(wk_sb[:], wk_d.ap().rearrange("(kt p) m -> p kt m", p=128))
        x_chunk(0, 0, 4)
        nc.sync.dma_start(wq_sb[:], wq_d.ap().rearrange("(kt p) m -> p kt m", p=128))
        x_chunk(0, 4, 8)
        nc.sync.dma_start(bqk_sb[:], bqk_d.ap())
        nc.sync.dma_start(wv_sb[:], wv_d.ap().rearrange("(kt p) m -> p kt m", p=128))
        nc.sync.dma_start(bv_sb[:], bv_d.ap())
        for sc in range(1, 4):
            x_chunk(sc, 0, 8)
        nc.sync.dma_start(wo_sb[:], wo_d.ap().rearrange("(kt p) m -> p kt m", p=128))

        ps = ctx.enter_context(tc.tile_pool(name="ps", bufs=2, space="PSUM"))
        op_ = ctx.enter_context(tc.tile_pool(name="op", bufs=3, space="PSUM"))
        fp = ctx.enter_context(tc.tile_pool(name="fp", bufs=1, space="PSUM"))
        ep = ctx.enter_context(tc.tile_pool(name="ep", bufs=12))
        rp = ctx.enter_context(tc.tile_pool(name="rp", bufs=3))
        tp = ctx.enter_context(tc.tile_pool(name="tp", bufs=3))
        osb = ctx.enter_context(tc.tile_pool(name="osb", bufs=3))

        # ---- filler groups, split into ~2-matmul parts (shared psum tile
        # per group; parts of one group must be emitted consecutively
        # relative to other fp-pool groups since fp has a single slot).
        def qk_group(proj, mt, c):
            """q (proj=0) / k (proj=1) projection of one 512-col chunk of
            head-pair mt, as 4 parts of 2 k-tiles each."""
            g = {}
            w_sb = wq_sb if proj == 0 else wk_sb

            def part(k0, k1):
                def f():
                    if "p" not in g:
                        g["p"] = fp.tile([128, 512], f32, tag="f", name="qkf")
                    p = g["p"]
                    for kt in range(k0, k1):
                        nc.tensor.matmul(
                            p[:],
                            w_sb[:, kt, mt * 128 : (mt + 1) * 128],
                            xbf[:, kt, c * 512 : (c + 1) * 512],
                            start=(kt == 0),
                            stop=(kt == KT - 1),
                        )
                    if k1 == KT:
                        dst = (qT if proj == 0 else kT)[:, mt, c * 512 : (c + 1) * 512]
                        bias_ap = bqk_sb[:, proj * 2 + mt : proj * 2 + mt + 1]
                        if proj == 0:
                            nc.vector.tensor_scalar(
                                dst, p[:], bias_ap, 0.125, Alu.add, Alu.mult
                            )
                        else:
                            nc.vector.tensor_scalar(dst, p[:], bias_ap, None, Alu.add)

                return f

            return [part(0, 2), part(2, 4), part(4, 6), part(6, 8)]

        def v_group(st):
            g = {}

            def part(k0, k1):
                def f():
                    if "p" not in g:
                        g["p"] = fp.tile([128, 512], f32, tag="f", name="vf")
                    p = g["p"]
                    for kt in range(k0, k1):
                        nc.tensor.matmul(
                            p[:, 0:DL],
                            xbf[:, kt, st * 128 : (st + 1) * 128],
                            wv_sb[:, kt, :],
                            start=(kt == 0),
                            stop=(kt == KT - 1),
                        )
                    if k1 == KT:
                        nc.vector.tensor_tensor(
                            vaug[:, st, :, 0:DH],
                            p[:, 0:DL].rearrange("p (h d) -> p h d", h=HPC),
                            bv_sb[:].rearrange("p (h d) -> p h d", h=HPC),
                            Alu.add,
                        )

                return f

            return [part(0, 2), part(2, 4), part(4, 6), part(6, 8)]

        def o_part(st, oc):
            """One 512-col chunk of the output projection for s-tile st:
            2 matmuls + evict + store, self-contained."""

            def f():
                pso = fp.tile([128, 512], f32, tag="f")
                for kt2 in range(2):
                    nc.tensor.matmul(
                        pso[:],
                        aoT[:, kt2, st * 128 : (st + 1) * 128],
                        wo_sb[:, kt2, oc * 512 : (oc + 1) * 512],
                        start=(kt2 == 0),
                        stop=(kt2 == 1),
                    )
                stg = osb.tile([128, 512], bf16, tag="oh")
                nc.vector.tensor_copy(stg[:], pso[:])
                nc.sync.dma_start(
                    out_d.ap()[st * 128 : (st + 1) * 128, oc * 512 : (oc + 1) * 512],
                    stg[:],
                )

            return f

        def attn_norm(h, ic, O):
            pb, mt = 64 * (h % 2), h // 2
            den = rp.tile([1, IC], f32, tag="den")
            nc.vector.tensor_copy(den[:], O[DH : DH + 1, :])
            recip = rp.tile([1, IC], f32, tag="r")
            nc.vector.reciprocal_approx_fast(recip[:], den[:])
            rb = rp.tile([64, IC], f32, tag="rb")
            nc.gpsimd.partition_broadcast(rb[:], recip[:])
            tmp = tp.tile([64, IC], bf16, tag="t")
            nc.vector.tensor_tensor(tmp[:], O[0:DH, :], rb[:], Alu.mult)
            nc.sync.dma_start(aoT[pb : pb + 64, mt, ic * IC : (ic + 1) * IC], tmp[:])

        def pair_ic(pair, ic, fillers, defer_tail=0):
            """Attention for head pair (2*pair, 2*pair+1) on query chunk ic.
            fillers: {jt: [part, ...]} emitted inside that jt step.  PV runs
            one jt behind the exp so the in-order PE queue never waits on
            the ScalarE semaphore."""
            hA, hB = 2 * pair, 2 * pair + 1
            OA = op_.tile([128, IC], f32, tag="O")
            OB = op_.tile([128, IC], f32, tag="O")

            def pv(jt, E):
                nc.tensor.matmul(
                    OA[:], vaug[:, jt, hA, :], E[:, 0:IC],
                    start=(jt == 0), stop=(jt == ST - 1),
                )
                nc.tensor.matmul(
                    OB[:], vaug[:, jt, hB, :], E[:, IC : 2 * IC],
                    start=(jt == 0), stop=(jt == ST - 1),
                )

            deferred = []
            prevE = None
            for jt in range(ST):
                Sp = ps.tile([128, 2 * IC], f32, tag="S")
                nc.tensor.matmul(
                    Sp[:, 0:IC],
                    kT[0:64, pair, jt * 128 : (jt + 1) * 128],
                    qT[0:64, pair, ic * IC : (ic + 1) * IC],
                    start=True, stop=True,
                )
                nc.tensor.matmul(
                    Sp[:, IC : 2 * IC],
                    kT[64:128, pair, jt * 128 : (jt + 1) * 128],
                    qT[64:128, pair, ic * IC : (ic + 1) * IC],
                    start=True, stop=True,
                )
                E = ep.tile([128, 2 * IC], bf16, tag="E")
                nc.scalar.activation(E[:], Sp[:], Act.Exp)
                if prevE is not None:
                    if jt - 1 >= ST - defer_tail:
                        deferred.append(lambda j=jt - 1, Ep=prevE: pv(j, Ep))
                    else:
                        pv(jt - 1, prevE)
                for f in fillers.get(jt, ()):
                    f()
                prevE = E
            tailwork = [
                lambda Ep=prevE: pv(ST - 1, Ep),
                lambda: attn_norm(hA, ic, OA),
                lambda: attn_norm(hB, ic, OB),
            ]
            if defer_tail:
                deferred.extend(tailwork)
                return deferred
            for f in tailwork:
                f()
            return []

        # ---- emission schedule ----
        # Head (inside the DMA window): only kT(0,0) + qT(0,0) so the first
        # scores/exp fire as early as possible; everything else streams
        # through the jt filler slots.
        for f in qk_group(1, 0, 0):
            f()
        for f in qk_group(0, 0, 0):
            f()

        def slots(*assign):
            """assign: list of (slot, [parts...]) -> fillers dict."""
            d = {}
            for slot, parts in assign:
                d.setdefault(slot, []).extend(parts)
            return d

        K01, K02, K03 = qk_group(1, 0, 1), qk_group(1, 0, 2), qk_group(1, 0, 3)
        Q01 = qk_group(0, 0, 1)
        V = {st: v_group(st) for st in range(ST)}

        # phase 0 = (0,0): all 16 v groups (v(st) fully emitted by the end
        # of slot st; early is fine) + kT(0,1..3) ahead of the scores that
        # read them + qT(0,1) for phase 1.  Packed into slots 0-13 so the
        # last slots carry no fillers and phase 1's scores/exp can start
        # immediately; the last two PVs + norms are deferred into phase 1.
        ph0 = slots(
            (0, V[0] + V[1]),
            (1, V[2] + [K01[0], K01[1]]),
            (2, V[3] + [K01[2], K01[3]]),
            (3, V[4]),
            (4, V[5]),
            (5, V[6] + [K02[0], K02[1]]),
            (6, V[7] + [K02[2], K02[3]]),
            (7, V[8] + [V[9][0], V[9][1]]),
            (8, V[9][2:] + [V[10][0], V[10][1]]),
            (9, V[10][2:] + [K03[0], K03[1]]),
            (10, V[11] + [K03[2], K03[3]]),
            (11, V[12] + [V[13][0], V[13][1]]),
            (12, V[13][2:] + V[14] + [Q01[0], Q01[1]]),
            (13, V[15] + [Q01[2], Q01[3]]),
        )
        d0 = pair_ic(0, 0, ph0, defer_tail=2)

        K10, Q10 = qk_group(1, 1, 0), qk_group(0, 1, 0)
        # phase 1 = (0,1): phase 0's deferred tail first, then kT(1,0) +
        # qT(1,0) for phase 2.
        ph1 = slots(
            (0, d0),
            (1, [K10[0]]), (2, [K10[1]]), (3, [K10[2]]), (4, [K10[3]]),
            (5, [Q10[0]]), (6, [Q10[1]]), (7, [Q10[2]]), (8, [Q10[3]]),
        )
        pair_ic(0, 1, ph1)

        K11, K12, K13 = qk_group(1, 1, 1), qk_group(1, 1, 2), qk_group(1, 1, 3)
        Q11 = qk_group(0, 1, 1)
        # phase 2 = (1,0): remaining kT(1,*) just-in-time + qT(1,1).
        ph2 = slots(
            (0, [K11[0], K11[1]]),
            (1, [K11[2], K11[3]]),
            (2, [Q11[0]]), (3, [Q11[1]]),
            (4, [K12[0], K12[1]]),
            (5, [K12[2], K12[3]]),
            (6, [Q11[2]]), (7, [Q11[3]]),
            (8, [K13[0], K13[1]]),
            (9, [K13[2], K13[3]]),
        )
        pair_ic(1, 0, ph2)

        Q02 = qk_group(0, 0, 2)
        # phase 3 = (1,1): qT(0,2) + o_proj of ic0 s-tiles.
        ph3 = slots(
            (0, [Q02[0]]), (1, [Q02[1]]), (2, [Q02[2]]), (3, [Q02[3]]),
            (4, [o_part(0, 0)]), (5, [o_part(0, 1)]),
            (6, [o_part(1, 0)]), (7, [o_part(1, 1)]),
            (8, [o_part(2, 0)]), (9, [o_part(2, 1)]),
            (10, [o_part(3, 0)]), (11, [o_part(3, 1)]),
        )
        pair_ic(1, 1, ph3)

        Q12 = qk_group(0, 1, 2)
        ph4 = slots(
            (0, [Q12[0]]), (1, [Q12[1]]), (2, [Q12[2]]), (3, [Q12[3]]),
            (4, [o_part(4, 0)]), (5, [o_part(4, 1)]),
            (6, [o_part(5, 0)]), (7, [o_part(5, 1)]),
        )
        pair_ic(0, 2, ph4)

        Q03 = qk_group(0, 0, 3)
        ph5 = slots(
            (0, [Q03[0]]), (1, [Q03[1]]), (2, [Q03[2]]), (3, [Q03[3]]),
            (4, [o_part(6, 0)]), (5, [o_part(6, 1)]),
            (6, [o_part(7, 0)]), (7, [o_part(7, 1)]),
        )
        pair_ic(1, 2, ph5)

        Q13 = qk_group(0, 1, 3)
        ph6 = slots(
            (0, [Q13[0]]), (1, [Q13[1]]), (2, [Q13[2]]), (3, [Q13[3]]),
            (4, [o_part(8, 0)]), (5, [o_part(8, 1)]),
            (6, [o_part(9, 0)]), (7, [o_part(9, 1)]),
        )
        pair_ic(0, 3, ph6)

        ph7 = slots(
            (1, [o_part(10, 0)]), (2, [o_part(10, 1)]),
            (3, [o_part(11, 0)]), (4, [o_part(11, 1)]),
        )
        pair_ic(1, 3, ph7)

        # ---- dense tail: o_proj for s-tiles 12..15 through the freed Sp
        # pool (2x 2-bank tiles) so matmuls pipeline while evicts/DMAs run.
        def o_tail_mm(st):
            pso = ps.tile([128, 2 * IC], f32, tag="S")
            for n in range(2):
                for kt2 in range(2):
                    nc.tensor.matmul(
                        pso[:, n * 512 : (n + 1) * 512],
                        aoT[:, kt2, st * 128 : (st + 1) * 128],
                        wo_sb[:, kt2, n * 512 : (n + 1) * 512],
                        start=(kt2 == 0),
                        stop=(kt2 == 1),
                    )
            return pso

        def o_tail_evict(st, pso):
            for n in range(2):
                stg = osb.tile([128, 512], bf16, tag="oh")
                nc.vector.tensor_copy(stg[:], pso[:, n * 512 : (n + 1) * 512])
                nc.sync.dma_start(
                    out_d.ap()[st * 128 : (st + 1) * 128, n * 512 : (n + 1) * 512],
                    stg[:],
                )

        p12 = o_tail_mm(12)
        p13 = o_tail_mm(13)
        o_tail_evict(12, p12)
        p14 = o_tail_mm(14)
        o_tail_evict(13, p13)
        p15 = o_tail_mm(15)
        o_tail_evict(14, p14)
        o_tail_evict(15, p15)

    nc.compile()
    return nc


def _get_nc():
    global _nc_cache
    if _nc_cache is None:
        _nc_cache = _build_nc()
    return _nc_cache


def _prepare_in_maps(x, W_q, b_q, W_k, b_k, W_v, b_v, W_o, b_o):
    in_maps = []
    for c in range(N_CORES):
        b, g = c // 4, c % 4
        rows = slice(DL * g, DL * g + DL)
        bqk = np.stack(
            [
                b_q[DL * g : DL * g + 128],
                b_q[DL * g + 128 : DL * g + 256],
                b_k[DL * g : DL * g + 128],
                b_k[DL * g + 128 : DL * g + 256],
            ],
            axis=1,
        ).astype(np.float32)
        in_maps.append(
            {
                "xT": np.ascontiguousarray(x[b].T).astype(_BF16),
                "wq": np.ascontiguousarray(W_q[rows].T).astype(_BF16),
                "wk": np.ascontiguousarray(W_k[rows].T).astype(_BF16),
                "wv": np.ascontiguousarray(W_v[rows].T).astype(_BF16),
                "wo": np.ascontiguousarray(W_o[:, rows].T).astype(_BF16),
                "bqk": np.ascontiguousarray(bqk),
                "bv": np.ascontiguousarray(
                    np.broadcast_to(b_v[rows], (128, DL))
                ).astype(np.float32),
            }
        )
    return in_maps


def _assemble(results, b_o):
    out = np.empty((B, S, D), dtype=np.float32)
    for b in range(B):
        acc = results[4 * b]["out"].astype(np.float32)
        for g in range(1, 4):
            acc += results[4 * b + g]["out"].astype(np.float32)
        out[b] = acc + b_o[None, :].astype(np.float32)
    return out


def kernel(x, W_q, b_q, W_k, b_k, W_v, b_v, W_o, b_o):
    from concourse.bass_utils import run_bass_kernel_spmd

    x = np.asarray(x, dtype=np.float32)
    nc = _get_nc()
    in_maps = _prepare_in_maps(
        x,
        np.asarray(W_q, np.float32),
        np.asarray(b_q, np.float32),
        np.asarray(W_k, np.float32),
        np.asarray(b_k, np.float32),
        np.asarray(W_v, np.float32),
        np.asarray(b_v, np.float32),
        np.asarray(W_o, np.float32),
        np.asarray(b_o, np.float32),
    )
    res = run_bass_kernel_spmd(nc, in_maps, core_ids=list(range(N_CORES)))
    return _assemble(res.results, np.asarray(b_o, np.float32))
